# revision 55
# baseline (speedup 1.0000x reference)
"""DeltaModel Trainium2 kernel (v4).

Key observation: the normalized key vector k_t = LN(embed[v] + FFN(embed[v]))
is a pure function of the token id v (64 vocab entries), so the whole
front-end (embedding, FFN, LayerNorm, per-token beta) collapses into a
64-row table computed on the host in f32.  The host gathers the per-token
keys (K token-major, K^T and (beta*K)^T feature-major) and ships them to
SBUF via DMA; the device only runs the chunked delta-rule recurrence, and
the readout (q^T M Wr Wo + bias) happens on the host from the DMA'd final
fast-weight matrix.

Math per 128-token chunk (A = strict_lower(Kb K^T), T = (I+A)^{-1}):
  W = T K, Z = T Kb ~= s*W   (s = mean beta; per-token beta deviates <0.1%)
  mt' = mt + K^T W - s (W^T K)^T mt     (mt = M^T)
T is applied via a 32-block split: T_bd ~= (I - A_bd)(I + A_bd^2)
(BD_TERMS=4; =6 adds the A^4 Horner pass), then the exact outer
correction (I + N)^{-1} = I - N + N^2 - N^3 (N = T_bd A_off, nilpotent,
only columns 0:96 nonzero) as 3 Horner stages.  N^T is built directly
from U2n via N^T = U2n^T (I - A_bd^T) - no separate token-major N.

Performance structure (CoreSim cost model): engine-op cost keys on the
free-axis length only plus a fixed per-op overhead, so all per-chunk
matrices for (2 chunks x 2 batch) = 4 units are stacked along the free
axis of shared [128, 4, *] tiles.  PSUM tiles are grouped into pools by
lifetime (A/AT -> solve -> V/state) so the 8 banks sustain ~4 pairs in
flight.  Masked extracts run on the otherwise-idle Pool engine
(SBUF-only); PSUM->SBUF copies are balanced between Act and DVE ("X +
psum" ops are either PE identity-matmul + Act copy, or DVE
tensor_tensor).  Matmuls are all bf16 moving operands (1 cyc/row).
"""

import numpy as np

H = 64
V = 64
B = 16
L = 2048
NCORES = 8
BPC = B // NCORES          # batch per core = 2
C = 128                    # chunk length
NCH = L // C               # 16 chunks (key 2047 zero-padded)
NPAIR = NCH // 2           # chunk pairs, 4 stacked units each
LN_EPS = 1e-5
D_EPS = 1e-6

BD_TERMS = 4

_CACHE = {}


def _build_nc(s_const, legalize=True):
    import concourse.bass as bass
    import concourse.mybir as mybir
    import concourse.tile as tile
    from concourse import masks

    dt = mybir.dt
    f32 = dt.float32
    f32r = dt.float32r
    bf16 = dt.bfloat16
    Alu = mybir.AluOpType
    Act = mybir.ActivationFunctionType

    nc = bass.Bass()

    kt_p = nc.declare_dram_parameter("kt", [64, NCH, BPC, C], bf16, isOutput=False)
    kbt_p = nc.declare_dram_parameter("kbt", [64, NCH, BPC, C], bf16, isOutput=False)
    k_p = nc.declare_dram_parameter("k", [C, NCH, BPC, H], bf16, isOutput=False)
    out_p = nc.declare_dram_parameter("out", [H, BPC, H], f32r, isOutput=True)

    from contextlib import ExitStack
    with tile.TileContext(nc) as tc, ExitStack() as est:
        persist = est.enter_context(tc.tile_pool(name="persist", bufs=1))

        def _tile(shape, dtype, name):
            return persist.tile(shape, dtype, name=name, tag=name)

        # ---------- constants ----------
        If32 = _tile([128, 128], f32, "If32")
        masks.make_identity(nc, If32[:])
        I128b = _tile([128, 128], bf16, "I128b")
        nc.vector.tensor_copy(I128b[:], If32[:])
        negI128b = _tile([128, 128], bf16, "negI128b")
        nc.gpsimd.tensor_scalar_mul(negI128b[:], I128b[:], -1.0)

        # f32 staging masks (strict lower / neg strict upper in 32-blocks,
        # off-block lower for cols < 96)
        mbd = _tile([128, 128], f32, "mbd")
        nc.gpsimd.memset(mbd[:], 0.0)
        for blk in range(4):
            sub = mbd[32 * blk:32 * blk + 32, 32 * blk:32 * blk + 32]
            nc.gpsimd.affine_select(
                out=sub, in_=sub, compare_op=Alu.is_ge, fill=1.0,
                base=0, pattern=[[1, 32]], channel_multiplier=-1)
        mup = _tile([128, 128], f32, "mup")
        nc.gpsimd.memset(mup[:], 0.0)
        for blk in range(4):
            sub = mup[32 * blk:32 * blk + 32, 32 * blk:32 * blk + 32]
            nc.gpsimd.affine_select(
                out=sub, in_=sub, compare_op=Alu.is_ge, fill=-1.0,
                base=0, pattern=[[-1, 32]], channel_multiplier=1)
        moff = _tile([128, 96], f32, "moff")
        nc.gpsimd.memset(moff[:], 0.0)
        for jb in range(3):
            for ib in range(jb + 1, 4):
                nc.gpsimd.memset(
                    moff[32 * ib:32 * ib + 32, 32 * jb:32 * jb + 32], 1.0)

        bdmask4 = _tile([128, 4, 128], bf16, "bdmask4")
        numask4 = _tile([128, 4, 128], bf16, "numask4")
        offmask4 = _tile([128, 4, 96], bf16, "offmask4")
        for u in range(4):
            nc.gpsimd.tensor_copy(bdmask4[:, u, :], mbd[:])
            nc.vector.tensor_copy(numask4[:, u, :], mup[:])
            nc.scalar.copy(offmask4[:, u, :], moff[:])

        # ---------- input key tables ----------
        KTt = _tile([64, NCH, BPC, C], bf16, "KTt")
        KbTt = _tile([64, NCH, BPC, C], bf16, "KbTt")
        Kt = _tile([C, NCH, BPC, H], bf16, "Kt")
        nc.sync.dma_start(KTt[:, 0:2, :, :], kt_p[:, 0:2, :, :])
        nc.scalar.dma_start(KbTt[:, 0:2, :, :], kbt_p[:, 0:2, :, :])
        for lo, hi in [(2, 4), (4, 6), (6, 10), (10, NCH)]:
            nc.sync.dma_start(KTt[:, lo:hi, :, :], kt_p[:, lo:hi, :, :])
            nc.sync.dma_start(KbTt[:, lo:hi, :, :], kbt_p[:, lo:hi, :, :])
        nc.gpsimd.dma_start(Kt[:, 0:4, :, :], k_p[:, 0:4, :, :])
        nc.gpsimd.dma_start(Kt[:, 4:8, :, :], k_p[:, 4:8, :, :])
        nc.gpsimd.dma_start(Kt[:, 8:NCH, :, :], k_p[:, 8:NCH, :, :])

        def KT(c, b):
            return KTt[:, c, b, :]

        def KbT(c, b):
            return KbTt[:, c, b, :]

        # ---------- pools ----------
        # psum tiles grouped by lifetime so the rings stay deep:
        #  psA: tP1 (A), tP2 (AT) - freed right after their masked copies
        #  psM: tP3 (S2/U1k/U2k/X3y), tP4 (U1n/U2n/NT)
        #  psV: tP5 (V1/V2/V3/zk/St)
        psA = est.enter_context(tc.tile_pool(name="psA", bufs=2, space="PSUM"))
        psM = est.enter_context(tc.tile_pool(name="psM", bufs=4, space="PSUM"))
        psV = est.enter_context(tc.tile_pool(name="psV", bufs=2, space="PSUM"))
        sb_af = est.enter_context(tc.tile_pool(name="sb_af", bufs=5))
        sb_m = est.enter_context(tc.tile_pool(name="sb_m", bufs=7))
        sb_u = est.enter_context(tc.tile_pool(name="sb_u", bufs=7))
        sb_x = est.enter_context(tc.tile_pool(name="sb_x", bufs=7))
        sb_v = est.enter_context(tc.tile_pool(name="sb_v", bufs=7))
        sb_mt = est.enter_context(tc.tile_pool(name="sb_mt", bufs=4))

        mt_cur = [None]
        P = [dict() for _ in range(NPAIR)]

        def units_of(cc):
            c0 = 2 * cc
            return [(c0, 0), (c0, 1), (c0 + 1, 0), (c0 + 1, 1)]

        # Stages of one pair, emitted in software-pipelined waves so each
        # engine's in-order instruction stream interleaves pairs.  The A and
        # posNT psum->sbuf copies ride the (otherwise idle) DMA engines.
        def s0(cc, t):
            t["tP1"] = tP1 = psA.tile([128, 4, 128], f32, name="tP1", tag="PA")
            t["tP2"] = tP2 = psA.tile([128, 4, 128], f32, name="tP2", tag="PA")
            for u, (c, b) in enumerate(units_of(cc)):
                nc.tensor.matmul(tP1[:, u, :], lhsT=KbT(c, b),
                                 rhs=KT(c, b), start=True, stop=True)
                nc.tensor.matmul(tP2[:, u, :], lhsT=KT(c, b),
                                 rhs=KbT(c, b), start=True, stop=True)

        def s1(cc, t):
            t["Acp"] = Acp = sb_af.tile([128, 4, 128], bf16, name="Acp")
            nc.scalar.copy(Acp[:], t["tP1"][:])
            t["Sbd"] = Sbd = sb_m.tile([128, 4, 128], bf16, name="Sbd4")
            nc.vector.tensor_mul(Sbd[:], t["tP2"][:], numask4[:])  # -(A_bd)^T

        def s2(cc, t):
            t["Abd"] = Abd = sb_m.tile([128, 4, 128], bf16, name="Abd4")
            nc.gpsimd.tensor_mul(Abd[:], t["Acp"][:], bdmask4[:])
            t["Aoff"] = Aoff = sb_m.tile([128, 4, 96], bf16, name="Aoff4")
            nc.gpsimd.tensor_mul(Aoff[:], t["Acp"][:, :, 0:96], offmask4[:])

        def s3(cc, t):
            t["tP3"] = tP3 = psM.tile([128, 4, 128], f32, name="tP3", tag="PM")
            for u in range(4):
                nc.tensor.matmul(tP3[:, u, :], lhsT=t["Abd"][:, u, :],
                                 rhs=t["Sbd"][:, u, :], start=True, stop=True)
            t["S2"] = S2 = sb_m.tile([128, 4, 128], bf16, name="S2pos")
            nc.scalar.activation(S2[:], tP3[:], Act.Copy, scale=-1.0)

        def s4(cc, t):
            # BD_TERMS=4: U2 = (I + A^2) R directly; =6 adds the U1 pass
            tP3, S2, Aoff = t["tP3"], t["S2"], t["Aoff"]
            t["tP4"] = tP4 = psM.tile([128, 4, 128], f32, name="tP4", tag="PM")
            c0 = 2 * cc
            if BD_TERMS == 6:
                for u, (c, b) in enumerate(units_of(cc)):
                    nc.tensor.matmul(tP3[:, u, 0:64], lhsT=S2[:, u, :],
                                     rhs=Kt[:, c, b, :], start=True, stop=True)
                    nc.tensor.matmul(tP4[:, u, 0:96], lhsT=I128b[:],
                                     rhs=Aoff[:, u, :], start=True, stop=False)
                    nc.tensor.matmul(tP4[:, u, 0:96], lhsT=S2[:, u, :],
                                     rhs=Aoff[:, u, :], start=False, stop=True)
                t["U1k"] = U1k = sb_u.tile([128, 4, 64], bf16, name="U1k")
                nc.vector.tensor_add(U1k[:], Kt[:, c0:c0 + 2, :, :],
                                     tP3[:, :, 0:64])
                t["U1n"] = U1n = sb_u.tile([128, 4, 96], bf16, name="U1n")
                nc.scalar.copy(U1n[:], tP4[:, :, 0:96])

        def s5(cc, t):
            tP3, tP4, S2, Aoff = t["tP3"], t["tP4"], t["S2"], t["Aoff"]
            if BD_TERMS == 6:
                rk = lambda u, c, b: t["U1k"][:, u, :]
                rn = lambda u: t["U1n"][:, u, :]
            else:
                rk = lambda u, c, b: Kt[:, c, b, :]
                rn = lambda u: Aoff[:, u, :]
            for u, (c, b) in enumerate(units_of(cc)):
                nc.tensor.matmul(tP3[:, u, 64:128], lhsT=S2[:, u, :],
                                 rhs=rk(u, c, b), start=True, stop=True)
                nc.tensor.matmul(tP4[:, u, 0:96], lhsT=I128b[:],
                                 rhs=Aoff[:, u, :], start=True, stop=False)
                nc.tensor.matmul(tP4[:, u, 0:96], lhsT=S2[:, u, :],
                                 rhs=rn(u), start=False, stop=True)
            c0 = 2 * cc
            t["U2k"] = U2k = sb_u.tile([128, 4, 64], bf16, name="U2k")
            nc.vector.tensor_add(U2k[:], Kt[:, c0:c0 + 2, :, :],
                                 tP3[:, :, 64:128])
            t["U2n"] = U2n = sb_u.tile([128, 4, 96], bf16, name="U2n")
            nc.scalar.copy(U2n[:], tP4[:, :, 0:96])

        def s6(cc, t):
            tP3, tP4, Sbd, U2k, U2n = (t["tP3"], t["tP4"], t["Sbd"],
                                       t["U2k"], t["U2n"])
            on_act = cc % 2 == 1
            for u in range(4):
                if on_act:
                    nc.tensor.matmul(tP3[:, u, 0:64], lhsT=I128b[:],
                                     rhs=U2k[:, u, :], start=True, stop=False)
                nc.tensor.matmul(tP3[:, u, 0:64], lhsT=Sbd[:, u, :],
                                 rhs=U2k[:, u, :], start=not on_act, stop=True)
            t["X3y"] = X3y = sb_x.tile([128, 4, 64], bf16, name="X3y")
            if on_act:
                nc.scalar.copy(X3y[:], tP3[:, :, 0:64])
            else:
                nc.vector.tensor_add(X3y[:], U2k[:], tP3[:, :, 0:64])
            # negNT = -Ntil^T = -U2n^T (I - A_bd^T)
            for u in range(4):
                nc.tensor.matmul(tP4[0:96, u, :], lhsT=U2n[:, u, :],
                                 rhs=I128b[:], start=True, stop=False)
                nc.tensor.matmul(tP4[0:96, u, :], lhsT=U2n[:, u, :],
                                 rhs=Sbd[:, u, :], start=False, stop=True)
            t["NT"] = NT = sb_m.tile([128, 4, 128], bf16, name="negNT")
            nc.scalar.activation(NT[0:96, :, :], tP4[0:96, :, :], Act.Copy,
                                 scale=-1.0)

        # V-stage: either DVE (1 mm + tensor_add) or Act (id-mm + plain copy)
        def _vstage(cc, t, region, rhs_name, out_name, on_act):
            X3y, NT, tP5 = t["X3y"], t["NT"], t["tP5"]
            rhs = X3y if rhs_name == "X3y" else t[rhs_name]
            for u in range(4):
                if on_act:
                    nc.tensor.matmul(tP5[:, u, region], lhsT=I128b[:],
                                     rhs=X3y[:, u, :], start=True, stop=False)
                nc.tensor.matmul(tP5[:, u, region], lhsT=NT[0:96, u, :],
                                 rhs=rhs[0:96, u, :], start=not on_act,
                                 stop=True)
            t[out_name] = V = sb_v.tile([128, 4, 64], bf16, name=out_name)
            if on_act:
                nc.scalar.copy(V[:], tP5[:, :, region])
            else:
                nc.vector.tensor_add(V[:], X3y[:], tP5[:, :, region])

        def s7(cc, t):
            t["tP5"] = psV.tile([128, 4, 128], f32, name="tP5", tag="PV")
            _vstage(cc, t, slice(0, 64), "X3y", "V1", False)

        def s8(cc, t):
            _vstage(cc, t, slice(64, 128), "V1", "V2", cc % 2 == 1)

        def s9(cc, t):
            _vstage(cc, t, slice(0, 64), "V2", "V3", False)

        def s10(cc, t):
            tP5, V3 = t["tP5"], t["V3"]
            for u, (c, b) in enumerate(units_of(cc)):
                nc.tensor.matmul(tP5[0:64, u, 64:128], lhsT=V3[:, u, :],
                                 rhs=Kt[:, c, b, :], start=True, stop=True)
            t["negZK"] = negZK = sb_v.tile([64, 4, 64], f32r, name="negZK")
            nc.vector.tensor_scalar_mul(negZK[:], tP5[0:64, :, 64:128],
                                        -s_const)

        def _st(cc, t, half):
            tP5, V3 = t["tP5"], t["V3"]
            c0 = 2 * cc
            c = c0 + half
            first = (cc == 0 and half == 0)
            for b in range(BPC):
                u = 2 * half + b
                nc.tensor.matmul(tP5[0:64, u, 0:64], lhsT=Kt[:, c, b, :],
                                 rhs=V3[:, u, :], start=True, stop=first)
                if not first:
                    nc.tensor.matmul(tP5[0:64, u, 0:64],
                                     lhsT=t["negZK"][:, u, :],
                                     rhs=mt_cur[0][:, b, :],
                                     start=False, stop=True)
            mt_new = sb_mt.tile([64, BPC, 64], f32r, name="mt_new")
            if first:
                nc.vector.tensor_copy(mt_new[:], tP5[0:64, 0:2, 0:64])
            else:
                nc.vector.tensor_add(mt_new[:], mt_cur[0][:],
                                     tP5[0:64, 2 * half:2 * half + 2, 0:64])
            mt_cur[0] = mt_new

        def s11(cc, t):
            _st(cc, t, 0)

        def s12(cc, t):
            _st(cc, t, 1)

        def sALL(cc, t):
            for f in [s0, s1, s2, s3, s4, s5, s6, s7, s8, s9, s10, s11, s12]:
                f(cc, t)

        stages = [sALL]
        NS = len(stages)
        for wave in range(NPAIR + NS - 1):
            for k in range(NS - 1, -1, -1):
                cc = wave - k
                if 0 <= cc < NPAIR:
                    stages[k](cc, P[cc])
        nc.sync.dma_start(out_p[:, :, :], mt_cur[0][:, :, :])

    if legalize:
        _legalize_waits(nc, mybir)
    return nc


def _legalize_waits(nc, mybir):
    """This walrus build encodes at most one sync-wait per instruction.
    Split multi-wait instructions into single-wait NoOp prefixes on the
    same engine (engine queues execute in order, so semantics hold)."""
    k = 0
    for blk in nc.main_func.blocks:
        insts = blk.instructions
        out = []
        changed = False
        for inst in list(insts):
            si = inst.sync_info
            waits = list(si.on_wait) if si is not None and si.on_wait else []
            if len(waits) > 1:
                for w in waits[:-1]:
                    nop = mybir.InstNoOp(name=f"I-wsplit-{k}", ins=[], outs=[])
                    k += 1
                    nop.engine = inst.engine
                    nop.sync_info = mybir.SyncInfo(on_wait=[w], on_update=[])
                    out.append(nop)
                si.on_wait = [waits[-1]]
                changed = True
            out.append(inst)
        if changed:
            while len(insts):
                insts.pop()
            for x in out:
                insts.append(x)


def host_tables(inputs):
    """Per-vocab key table: k(v) = LN(embed[v] + FFN(embed[v])), f32."""
    g = lambda k: np.asarray(inputs[k], dtype=np.float64)
    emb = g("embed")
    ff = np.maximum(emb @ g("W1") + g("b1"), 0) @ g("W2") + g("b2")
    x = emb + ff
    mu = x.mean(-1, keepdims=True)
    var = x.var(-1, keepdims=True)
    ktab = ((x - mu) / np.sqrt(var + LN_EPS) * g("gamma") + g("beta"))
    ktab = ktab.astype(np.float32)
    beta_tab = (1.0 / ((ktab.astype(np.float64) ** 2).sum(-1) + D_EPS))
    beta_tab = beta_tab.astype(np.float32)
    kbtab = (ktab * beta_tab[:, None]).astype(np.float32)
    return ktab, beta_tab, kbtab


def core_inputs(seq_core, ktab, kbtab):
    """Gather per-core key tensors in the three DMA layouts (bf16)."""
    import ml_dtypes
    bf = ml_dtypes.bfloat16
    kg = np.zeros((BPC, L, H), np.float32)
    kbg = np.zeros((BPC, L, H), np.float32)
    kg[:, :L - 1] = ktab[seq_core[:, :L - 1]]
    kbg[:, :L - 1] = kbtab[seq_core[:, :L - 1]]
    kg4 = kg.reshape(BPC, NCH, C, H)
    kbg4 = kbg.reshape(BPC, NCH, C, H)
    # kt [64, NCH, BPC, C] : kt[f, c, b, t] = kg4[b, c, t, f]
    kt = np.ascontiguousarray(kg4.transpose(3, 1, 0, 2).astype(bf))
    kbt = np.ascontiguousarray(kbg4.transpose(3, 1, 0, 2).astype(bf))
    # k [C, NCH, BPC, H] : k[t, c, b, f] = kg4[b, c, t, f]
    k = np.ascontiguousarray(kg4.transpose(2, 1, 0, 3).astype(bf))
    return {"kt": kt, "kbt": kbt, "k": k}


def kernel(**inputs):
    from concourse.bass_utils import run_bass_kernel_spmd

    seq = np.ascontiguousarray(np.asarray(inputs["seq"], dtype=np.int64))
    ktab, beta_tab, kbtab = host_tables(inputs)
    s_const = float(beta_tab[seq[:, :L - 1]].mean())

    key = round(s_const, 10)
    if _CACHE.get("key") != key:
        _CACHE["nc"] = _build_nc(s_const)
        _CACHE["key"] = key
    nc = _CACHE["nc"]

    in_maps = [core_inputs(seq[core * BPC:(core + 1) * BPC], ktab, kbtab)
               for core in range(NCORES)]
    res = run_bass_kernel_spmd(nc, in_maps, core_ids=list(range(NCORES)))

    # host readout: y = (q^T mt) Wro + bias   (mt = M^T)
    g = lambda k: np.asarray(inputs[k], dtype=np.float32)
    Wro = g("Wr") @ g("Wo")
    bias = g("br") @ g("Wo") + g("bo")
    out = np.zeros((B, V), np.float32)
    for core in range(NCORES):
        mt = res.results[core]["out"]          # [64, BPC, 64] f32
        for b in range(BPC):
            gb = core * BPC + b
            q = ktab[seq[gb, L - 1]]
            ctx = mt[:, b, :].T @ q
            out[gb] = ctx @ Wro + bias
    return out.astype(np.float32)


if __name__ == "__main__":
    d = np.load("/root/problem/inputs.npz")
    y = kernel(**{k: d[k] for k in d.files})
    o = np.load("/root/problem/oracle.npz")
    rel = np.abs(y - o["y"]).max() / np.abs(o["y"]).max()
    print("Relative error:", rel)
